# revision 23
# baseline (speedup 1.0000x reference)
"""Multi-head attention (B=2, T=2048, D=1024, H=16, causal) on 8 Trainium2
NeuronCores.

Sharding: core c handles batch b = c//4 and head group g = c%4 (4 heads =
256 channels). Wq/Wk/Wv are column-parallel, Wo row-parallel; each core
produces a partial [T, D] output (fp16) and the host sums the 4 partials per
batch (the "all-reduce") and adds bo.

Per-core kernel (v2 — explicit static interleave):
  - All on-chip activations/weights in fp16 (psum accumulation fp32):
    fp16 LDWEIGHTS gets FWL (2x), DVE tensor_tensor runs 2x on 16-bit, and
    SBUF/DMA footprints halve.  Scores/exp stay well inside fp16 range.
  - Q^T/K^T projected into [128, pair, T] transposed layout from
    host-pretransposed x; V projected into Vaug[tk, tile, head, 128] where
    cols 0:64 are constant 1.0 and 64:128 are the V channels: the PV matmul
    xu[128, tq] += Vaug^T @ expS^T then yields the softmax denominator
    REPLICATED on psum rows 0:63 for free (PE-side broadcast), so
    normalization is just reciprocal_approx_fast + one tensor_tensor.
  - Scores transposed per 128x512 tile, both heads of a pair concurrent on
    PE row-groups; exp on ScalarE with the 1/sqrt(dh) scale folded in;
    0/1 mask multiply on diagonal tiles only (fp16, 2x DVE).
  - One DMA per K/Q block (1MB) and per V half-block (0.5MB) from
    host-packed layouts; output written per 128-row tile as fp16.
  - Emission is an explicit fine-grained static schedule: attention tiles
    are interleaved with "filler" units (projection chunk matmuls two
    blocks ahead, Wo tiles one block behind) so the PE never idles long
    enough for the HAM clock gate to re-throttle.
  - No exact softmax max-subtraction: scores ~N(0,8) pre-scale, exp never
    overflows fp16 after the 1/8 scale, masked lanes are exact zeros.
"""

import numpy as np
from contextlib import ExitStack

import concourse.bass as bass
import concourse.tile as tile
from concourse import bacc, mybir
from concourse.bass_utils import run_bass_kernel_spmd

F32 = mybir.dt.float32
F16 = mybir.dt.float16
EXP = mybir.ActivationFunctionType.Exp
MULT = mybir.AluOpType.mult
ADD = mybir.AluOpType.add

B, T, D, H = 2, 2048, 1024, 16
DH = D // H          # 64
HPC = H // 4         # 4 heads per core
DC = DH * HPC        # 256 channels per core
NBLK = T // 512      # 4 Tq blocks of 512
NT128 = T // 128     # 16 T tiles of 128
NCHUNK = D // 128    # 8 contraction chunks

_PROG = None


def _ensure_axon_hooks():
    """If the runtime sets BASS_TRACE, run_bass_kernel_spmd imports
    antenv.axon_hooks; provide a ctypes-backed NTFF hook when the real
    module isn't shipped (mirrors trn_agent_boot.trn_boot)."""
    try:
        import antenv.axon_hooks  # noqa: F401
        return
    except ImportError:
        pass
    import contextlib
    import ctypes
    import sys
    import types

    try:
        import antenv
    except ImportError:
        antenv = types.ModuleType("antenv")
        sys.modules["antenv"] = antenv

    def _build_hook():
        try:
            lib = ctypes.CDLL("/opt/axon/libaxon_pjrt.so")
        except OSError:
            return None
        if not hasattr(lib, "axon_start_nrt_profile"):
            return None
        lib.axon_start_nrt_profile.argtypes = [
            ctypes.POINTER(ctypes.c_int64),
            ctypes.c_size_t,
        ]
        lib.axon_start_nrt_profile.restype = ctypes.c_int64
        lib.axon_stop_nrt_profile.argtypes = [ctypes.c_char_p]
        lib.axon_stop_nrt_profile.restype = ctypes.c_int64

        @contextlib.contextmanager
        def _ntff_hook(output_dir, device_ids):
            import jax

            jax.devices()
            if device_ids:
                ids = (ctypes.c_int64 * len(device_ids))(*device_ids)
                rc = lib.axon_start_nrt_profile(ids, len(device_ids))
            else:
                rc = lib.axon_start_nrt_profile(None, 0)
            if rc != 0:
                raise RuntimeError(f"axon_start_nrt_profile rc={rc}")
            try:
                yield
            finally:
                n = lib.axon_stop_nrt_profile(str(output_dir).encode())
                if n < 0:
                    raise RuntimeError(f"axon_stop_nrt_profile rc={n}")

        return _ntff_hook

    mod = types.ModuleType("antenv.axon_hooks")
    _cell = {"hook": None, "built": False}

    def set_axon_ntff_profile_hook(hook):
        _cell["hook"] = hook
        _cell["built"] = True

    def get_axon_ntff_profile_hook():
        if not _cell["built"]:
            _cell["hook"] = _build_hook()
            _cell["built"] = True
        return _cell["hook"]

    mod.set_axon_ntff_profile_hook = set_axon_ntff_profile_hook
    mod.get_axon_ntff_profile_hook = get_axon_ntff_profile_hook
    sys.modules["antenv.axon_hooks"] = mod
    antenv.axon_hooks = mod


_ensure_axon_hooks()


def _build_program():
    nc = bacc.Bacc("TRN2", target_bir_lowering=False, debug=False)

    # host-packed inputs: xq/xk[blk, p, c, t] = x^T[128c+p, 512*blk+t]
    #                     xv[s, p, c, t]     = v^T[128c+p, 256*s+t]
    xq = nc.declare_dram_parameter("xq", [NBLK, 128, NCHUNK, 512], F16, isOutput=False)
    xk = nc.declare_dram_parameter("xk", [NBLK, 128, NCHUNK, 512], F16, isOutput=False)
    xv = nc.declare_dram_parameter("xv", [2 * NBLK, 128, NCHUNK, 256], F16, isOutput=False)
    wq = nc.declare_dram_parameter("wq", [128, NCHUNK, DC], F16, isOutput=False)
    wk = nc.declare_dram_parameter("wk", [128, NCHUNK, DC], F16, isOutput=False)
    wv = nc.declare_dram_parameter("wv", [128, NCHUNK, DC], F16, isOutput=False)
    wo = nc.declare_dram_parameter("wo", [128, 2, D], F16, isOutput=False)
    bqk = nc.declare_dram_parameter("bqk", [128, 4], F32, isOutput=False)
    maskp = nc.declare_dram_parameter("maskp", [128, 4, 512], F16, isOutput=False)
    outp = nc.declare_dram_parameter("outp", [NT128, 128, D], F16, isOutput=True)

    with tile.TileContext(nc) as tc, ExitStack() as ctx:
        # one pool per constant: each DMA gets its own semaphore slot so the
        # loads pipeline instead of serializing on completion round-trips
        cp_wv = ctx.enter_context(tc.tile_pool(name="c_wv", bufs=1))
        cp_wk = ctx.enter_context(tc.tile_pool(name="c_wk", bufs=1))
        cp_wq = ctx.enter_context(tc.tile_pool(name="c_wq", bufs=1))
        cp_wo = ctx.enter_context(tc.tile_pool(name="c_wo", bufs=1))
        cp_b = ctx.enter_context(tc.tile_pool(name="c_b", bufs=1))
        cp_m = ctx.enter_context(tc.tile_pool(name="c_m", bufs=1))
        persist = ctx.enter_context(tc.tile_pool(name="persist", bufs=1))
        xkqp = ctx.enter_context(tc.tile_pool(name="xkq", bufs=3))
        xvp = ctx.enter_context(tc.tile_pool(name="xv", bufs=3))
        esp = ctx.enter_context(tc.tile_pool(name="es", bufs=6))
        sop = ctx.enter_context(tc.tile_pool(name="so", bufs=3))
        rdp = ctx.enter_context(tc.tile_pool(name="rd", bufs=2))
        smp = ctx.enter_context(tc.tile_pool(name="small", bufs=1))
        stp = ctx.enter_context(tc.tile_pool(name="st", bufs=2, space="PSUM"))
        xup = ctx.enter_context(tc.tile_pool(name="xu", bufs=1, space="PSUM"))
        pwp = ctx.enter_context(tc.tile_pool(name="pw", bufs=2, space="PSUM"))

        # ---- constants (scalar HWDGE queue), in consumption order ----
        # wv split in half so the first V matmul can start ~2us earlier
        wv_sb = cp_wv.tile([128, NCHUNK, DC], F16)
        nc.scalar.dma_start(wv_sb[:, 0:4, :], wv[:, 0:4, :])
        nc.scalar.dma_start(wv_sb[:, 4:8, :], wv[:, 4:8, :])
        wk_sb = cp_wk.tile([128, NCHUNK, DC], F16)
        nc.scalar.dma_start(wk_sb[:], wk[:])
        wq_sb = cp_wq.tile([128, NCHUNK, DC], F16)
        nc.scalar.dma_start(wq_sb[:], wq[:])
        bqk_sb = cp_b.tile([128, 4], F32)
        nc.scalar.dma_start(bqk_sb[:], bqk[:])
        bq_sb = bqk_sb[:, 0:2]
        bk_sb = bqk_sb[:, 2:4]
        mask_sb = cp_m.tile([128, 4, 512], F16)
        nc.scalar.dma_start(mask_sb[:], maskp[:])
        wo_sb = cp_wo.tile([128, 2, D], F16)
        nc.scalar.dma_start(wo_sb[:], wo[:])

        # preload the exp table set early (one-time ~2.7us); feed it from a
        # memset tile so it does not wait on any DMA
        warm_in = smp.tile([1, 2], F32, tag="warmin")
        nc.vector.memset(warm_in[:], 1.0)
        warm = smp.tile([1, 2], F32, tag="warm")
        nc.scalar.activation(warm[:], warm_in[:], EXP, scale=1.0)

        # persistent activations
        QT = persist.tile([128, 2, T], F16)       # [2 heads x 64dh, pair, T]
        KT = persist.tile([128, 2, T], F16)
        XT = persist.tile([128, 2, T], F16)       # attention out, transposed
        # Vaug[tk, tile, head, 0:64] = 1.0 (denominator rows),
        # Vaug[tk, tile, head, 64:128] = V channels
        Vaug = persist.tile([128, NT128, HPC, 2 * DH], F16)
        nc.vector.memset(Vaug[:], 1.0)

        # ---------------- filler units ----------------
        filler = []  # list of (label, closure); popped strictly FIFO

        def pump(k):
            for _ in range(min(k, len(filler))):
                filler.pop(0)[1]()

        def drain(label):
            # emit every queued unit up to and including the last with `label`
            if not any(lb == label for lb, _ in filler):
                return
            while filler:
                lb, fn = filler.pop(0)
                fn()
                if lb == label and not any(l2 == label for l2, _ in filler):
                    break

        def drain_all():
            while filler:
                filler.pop(0)[1]()

        def v_units(s, vt):
            """V projection for T tiles 2s, 2s+1 -> Vaug[:, :, :, 64:128]."""
            st = {}

            def mk(c):
                def u():
                    if c == 0:
                        # separate psum tiles: start=True clears has_written
                        # BANK-wide, so the two halves must not share a bank
                        st[0] = pwp.tile([128, DC], F32, tag="pw", name="pv0")
                        st[1] = pwp.tile([128, DC], F32, tag="pw", name="pv1")
                    for k in (0, 1):
                        nc.tensor.matmul(
                            st[k][:], vt[:, c, 128 * k : 128 * (k + 1)],
                            wv_sb[:, c, :],
                            start=(c == 0), stop=(c == NCHUNK - 1),
                            skip_group_check=True,
                        )
                return u

            def fin():
                for k in (0, 1):
                    t = 2 * s + k
                    nc.vector.tensor_copy(
                        Vaug[:, t, :, DH : 2 * DH],
                        st[k][:].rearrange("p (h d) -> p h d", h=HPC),
                    )

            return [mk(c) for c in range(NCHUNK)] + [fin]

        def kq_units(xt, w_sb, b_sb, OUT, blk):
            """K^T or Q^T projection for Tq block blk (both pairs)."""
            st = {}

            def mk(c):
                def u():
                    if c == 0:
                        st[0] = pwp.tile([128, 512], F32, tag="pw", name="ps0")
                        st[1] = pwp.tile([128, 512], F32, tag="pw", name="ps1")
                    for p in (0, 1):
                        nc.tensor.matmul(
                            st[p][:], w_sb[:, c, 128 * p : 128 * (p + 1)],
                            xt[:, c, :],
                            start=(c == 0), stop=(c == NCHUNK - 1),
                            skip_group_check=True,
                        )
                return u

            def fin():
                for p in (0, 1):
                    nc.vector.tensor_scalar(
                        OUT[:, p, 512 * blk : 512 * (blk + 1)],
                        st[p][:], b_sb[:, p : p + 1], None, op0=ADD,
                    )

            return [mk(c) for c in range(NCHUNK)] + [fin]

        def wo_units(blk, copy_eng=None):
            """Output projection + store for the 4 T tiles of block blk."""
            st = {}
            units = []
            for t in range(4 * blk, 4 * blk + 4):
                for n in (0, 1):
                    def u(t=t, n=n):
                        if n == 0:
                            st[t] = sop.tile([128, D], F16, tag="so", name="so")
                        po = pwp.tile([128, 512], F32, tag="pw", name="po")
                        nc.tensor.matmul(
                            po[:], XT[:, 0, 128 * t : 128 * (t + 1)],
                            wo_sb[:, 0, 512 * n : 512 * (n + 1)],
                            start=True, stop=False, skip_group_check=True,
                        )
                        nc.tensor.matmul(
                            po[:], XT[:, 1, 128 * t : 128 * (t + 1)],
                            wo_sb[:, 1, 512 * n : 512 * (n + 1)],
                            start=False, stop=True, skip_group_check=True,
                        )
                        dst = st[t][:, 512 * n : 512 * (n + 1)]
                        if copy_eng == "scalar" or (copy_eng == "alt" and n == 1):
                            nc.scalar.copy(dst, po[:])
                        else:
                            nc.vector.tensor_copy(dst, po[:])
                        if n == 1:
                            nc.gpsimd.dma_start(outp[t], st[t][:])
                    units.append(u)
            return units

        # ---------------- attention ----------------
        last_es = [None]

        def att_tile(p, i, j, njt, xu):
            """One 128(tk) x 512(tq) score tile for head pair p, block i."""
            J = j - 4 * i
            c0 = 128 * J if J >= 0 else 0
            ps_t = stp.tile([128, 2, 512], F32, tag="st", name="ps_t")
            for hp in range(2):
                nc.tensor.matmul(
                    ps_t[:, hp, c0:512],
                    KT[64 * hp : 64 * hp + 64, p, 128 * j : 128 * (j + 1)],
                    QT[64 * hp : 64 * hp + 64, p, 512 * i + c0 : 512 * (i + 1)],
                    start=True, stop=True, skip_group_check=True,
                )
            es = esp.tile([128, 2, 512], F16, tag="es", name="es")
            nc.scalar.activation(
                es[:, :, c0:512], ps_t[:, :, c0:512], EXP,
                scale=1.0 / np.sqrt(DH),
            )
            if J >= 0:
                for hp in range(2):
                    nc.vector.tensor_tensor(
                        es[:, hp, c0:512], es[:, hp, c0:512],
                        mask_sb[:, J, c0:512], op=MULT,
                    )
            for hp in range(2):
                nc.tensor.matmul(
                    xu[:, hp, c0:512], Vaug[:, j, 2 * p + hp, :],
                    es[:, hp, c0:512],
                    start=(j == 0), stop=(j == njt - 1),
                    skip_group_check=True,
                )
            last_es[0] = es

        def att_normalize(p, i, xu):
            rd = rdp.tile([DH, 2, 512], F32, tag="rd", name="rd")
            nc.vector.reciprocal_approx_fast(rd[:], xu[0:DH, :, :])
            for hp in range(2):
                nc.vector.tensor_tensor(
                    XT[64 * hp : 64 * hp + 64, p, 512 * i : 512 * (i + 1)],
                    xu[DH:128, hp, :], rd[:, hp, :], op=MULT,
                )

        # ---------------- prologue ----------------
        # All x DMAs upfront on the sync HWDGE queue, in consumption order.
        # The bufs=2 pools gate them: block b's trigger fires once block
        # b-2's tiles have been consumed (self-pacing flow control).
        vts = {}
        kts = {}
        qts = {}
        vts[0] = xvp.tile([128, NCHUNK, 256], F16, tag="xv", name="vt")
        nc.sync.dma_start(vts[0][:, 0:4, :], xv[0][:, 0:4, :])
        nc.sync.dma_start(vts[0][:, 4:8, :], xv[0][:, 4:8, :])
        vts[1] = xvp.tile([128, NCHUNK, 256], F16, tag="xv", name="vt")
        nc.sync.dma_start(vts[1][:], xv[1])
        kts[0] = xkqp.tile([128, NCHUNK, 512], F16, tag="xk", name="kt")
        nc.sync.dma_start(kts[0][:, 0:4, :], xk[0][:, 0:4, :])
        nc.sync.dma_start(kts[0][:, 4:8, :], xk[0][:, 4:8, :])
        qts[0] = xkqp.tile([128, NCHUNK, 512], F16, tag="xq", name="qt")
        nc.sync.dma_start(qts[0][:, 0:4, :], xq[0][:, 0:4, :])
        nc.sync.dma_start(qts[0][:, 4:8, :], xq[0][:, 4:8, :])
        for b in (1, 2, 3):
            for s in (2 * b, 2 * b + 1):
                vts[s] = xvp.tile([128, NCHUNK, 256], F16, tag="xv", name="vt")
                nc.sync.dma_start(vts[s][:], xv[s])
            kts[b] = xkqp.tile([128, NCHUNK, 512], F16, tag="xk", name="kt")
            nc.sync.dma_start(kts[b][:], xk[b])
            qts[b] = xkqp.tile([128, NCHUNK, 512], F16, tag="xq", name="qt")
            nc.sync.dma_start(qts[b][:], xq[b])

        def proj_units(blk):
            return (
                [("proj%d" % blk, u) for u in v_units(2 * blk, vts[2 * blk])]
                + [("proj%d" % blk, u) for u in v_units(2 * blk + 1, vts[2 * blk + 1])]
                + [("proj%d" % blk, u) for u in kq_units(kts[blk], wk_sb, bk_sb, KT, blk)]
                + [("proj%d" % blk, u) for u in kq_units(qts[blk], wq_sb, bq_sb, QT, blk)]
            )

        # proj(0) drains inline (nothing else to do yet)
        for _, u in proj_units(0):
            u()
        filler.extend(proj_units(1))

        # ---------------- main loop over Tq blocks ----------------
        # filler placement: proj(i+1) during block i; wo(0) in b1 (XT(0)
        # ready), wo(1)+wo(2) deferred to b3 (the ACT-paced block with the
        # least projection filler)
        # wo(2) units double as pair-boundary cover in block 3: emitted
        # BEFORE each normalize (units emitted after a normalize inherit a
        # dependency on it through XT and cannot cover it), with psum->sbuf
        # copies on ScalarE so the normalize chain is not delayed on DVE
        wo2_cover = wo_units(2, copy_eng="scalar")

        for i in range(NBLK):
            if i == 1:
                filler.extend(proj_units(2))
                filler.extend([("wo0", u) for u in wo_units(0)])
            elif i == 2:
                filler.extend(proj_units(3))
            elif i == 3:
                filler.extend([("wo1", u) for u in wo_units(1)])

            # everything block i consumes must be emitted by now
            drain("proj%d" % i)

            njt = 4 * i + 4
            ntile = 2 * njt
            for p in (0, 1):
                xu = xup.tile([128, 2, 512], F32, tag="xu", name="xu")
                for j in range(njt):
                    att_tile(p, i, j, njt, xu)
                    # spread remaining filler over remaining attention tiles,
                    # holding back ~3 units per upcoming pair boundary to
                    # cover the normalize -> next-pair-PV psum dependency
                    rem_tiles = ntile - (p * njt + j + 1)
                    if rem_tiles > 0:
                        reserve = 3 * (2 - p)
                        avail = max(0, len(filler) - reserve)
                        need = (avail + rem_tiles - 1) // rem_tiles
                        pump(need)
                # boundary cover: emitted BEFORE the normalize so the PE
                # stays busy while the DVE normalize chain runs
                if i == 3:
                    ncov = 4 if p == 0 else len(wo2_cover)
                    for u in wo2_cover[:ncov]:
                        u()
                    del wo2_cover[:ncov]
                else:
                    pump(3)
                att_normalize(p, i, xu)
                if i == 3 and p == 1:
                    # HAM keep-warm: the final normalize leaves the PE with
                    # no legal work for ~2.7us, which re-throttles the clock
                    # and makes the whole wo(3) tail run at 1.2 GHz.  Pin
                    # dummy matmuls here — gated on the last es tile so the
                    # list scheduler cannot hoist them earlier.
                    for _ in range(12):
                        dps = stp.tile([128, 2, 512], F32, tag="st", name="dummy")
                        nc.tensor.matmul(
                            dps[:, 0, :], Vaug[:, 0, 0, :], last_es[0][:, 0, :],
                            start=True, stop=True, skip_group_check=True,
                        )

        drain_all()
        # tail: alternate the psum->sbuf copies DVE/ACT (ACT is idle here)
        for u in wo_units(3, copy_eng="alt"):
            u()

    nc.compile()
    return nc


def _get_program():
    global _PROG
    if _PROG is None:
        _PROG = _build_program()
    return _PROG


def _make_mask():
    r = np.arange(128)[:, None]
    c = np.arange(512)[None, :]
    m = np.zeros((128, 4, 512), np.float16)
    for J in range(4):
        m[:, J, :] = (c >= 128 * J + r).astype(np.float16)
    return m


def _core_inputs(inputs, b, g):
    """Per-core input map (host-side sharding/layout prep)."""
    f = np.float32
    sl = slice(DC * g, DC * (g + 1))
    wqp = np.ascontiguousarray(
        np.asarray(inputs["Wq"], f)[:, sl].reshape(NCHUNK, 128, DC).transpose(1, 0, 2)
    ).astype(np.float16)
    wkp = np.ascontiguousarray(
        np.asarray(inputs["Wk"], f)[:, sl].reshape(NCHUNK, 128, DC).transpose(1, 0, 2)
    ).astype(np.float16)
    wvp = np.ascontiguousarray(
        np.asarray(inputs["Wv"], f)[:, sl].reshape(NCHUNK, 128, DC).transpose(1, 0, 2)
    ).astype(np.float16)
    wop = np.ascontiguousarray(
        np.asarray(inputs["Wo"], f)[sl, :].reshape(2, 128, D).transpose(1, 0, 2)
    ).astype(np.float16)
    bqp = np.asarray(inputs["bq"], f)[sl].reshape(2, 128).T
    bkp = np.asarray(inputs["bk"], f)[sl].reshape(2, 128).T
    return {
        "wq": wqp, "wk": wkp, "wv": wvp, "wo": wop,
        "bqk": np.ascontiguousarray(np.concatenate([bqp, bkp], axis=1)),
        "maskp": _make_mask(),
    }


def _pack_x(xt_f16, inner):
    """[1024, 2048] fp16 x^T -> [T//inner, 128, 8, inner] host-packed."""
    return np.ascontiguousarray(
        xt_f16.reshape(NCHUNK, 128, T // inner, inner).transpose(2, 1, 0, 3)
    )


def run_cores(inputs, trace=False, trace_cores=None):
    nc = _get_program()
    f = np.float32
    xb = {}
    for b in range(B):
        qT = np.asarray(inputs["q"], f)[b].T.astype(np.float16)
        kT = np.asarray(inputs["k"], f)[b].T.astype(np.float16)
        vT = np.asarray(inputs["v"], f)[b].T.astype(np.float16)
        xb[b] = {
            "xq": _pack_x(qT, 512),
            "xk": _pack_x(kT, 512),
            "xv": _pack_x(vT, 256),
        }
    in_maps = []
    for c in range(8):
        b, g = divmod(c, 4)
        m = _core_inputs(inputs, b, g)
        m.update(xb[b])
        in_maps.append(m)
    kw = {}
    if trace:
        kw = dict(trace=True, trace_cores=trace_cores or [0])
    res = run_bass_kernel_spmd(nc, in_maps, list(range(8)), **kw)
    bo = np.asarray(inputs["bo"], f)
    out = np.empty((B, T, D), f)
    for b in range(B):
        acc = res.results[4 * b]["outp"].astype(f)
        for g in range(1, 4):
            acc = acc + res.results[4 * b + g]["outp"].astype(f)
        out[b] = acc.reshape(T, D) + bo
    return out, res


def kernel(**inputs) -> np.ndarray:
    out, _ = run_cores(inputs)
    return out


# revision 24
# speedup vs baseline: 1.0140x; 1.0140x over previous
"""Multi-head attention (B=2, T=2048, D=1024, H=16, causal) on 8 Trainium2
NeuronCores.

Sharding: core c handles batch b = c//4 and head group g = c%4 (4 heads =
256 channels). Wq/Wk/Wv are column-parallel, Wo row-parallel; each core
produces a partial [T, D] output (fp16) and the host sums the 4 partials per
batch (the "all-reduce") and adds bo.

Per-core kernel (v2 — explicit static interleave):
  - All on-chip activations/weights in fp16 (psum accumulation fp32):
    fp16 LDWEIGHTS gets FWL (2x), DVE tensor_tensor runs 2x on 16-bit, and
    SBUF/DMA footprints halve.  Scores/exp stay well inside fp16 range.
  - Q^T/K^T projected into [128, pair, T] transposed layout from
    host-pretransposed x; V projected into Vaug[tk, tile, head, 128] where
    cols 0:64 are constant 1.0 and 64:128 are the V channels: the PV matmul
    xu[128, tq] += Vaug^T @ expS^T then yields the softmax denominator
    REPLICATED on psum rows 0:63 for free (PE-side broadcast), so
    normalization is just reciprocal_approx_fast + one tensor_tensor.
  - Scores transposed per 128x512 tile, both heads of a pair concurrent on
    PE row-groups; exp on ScalarE with the 1/sqrt(dh) scale folded in;
    0/1 mask multiply on diagonal tiles only (fp16, 2x DVE).
  - One DMA per K/Q block (1MB) and per V half-block (0.5MB) from
    host-packed layouts; output written per 128-row tile as fp16.
  - Emission is an explicit fine-grained static schedule: attention tiles
    are interleaved with "filler" units (projection chunk matmuls two
    blocks ahead, Wo tiles one block behind) so the PE never idles long
    enough for the HAM clock gate to re-throttle.
  - No exact softmax max-subtraction: scores ~N(0,8) pre-scale, exp never
    overflows fp16 after the 1/8 scale, masked lanes are exact zeros.
"""

import numpy as np
from contextlib import ExitStack

import concourse.bass as bass
import concourse.tile as tile
from concourse import bacc, mybir
from concourse.bass_utils import run_bass_kernel_spmd

F32 = mybir.dt.float32
F16 = mybir.dt.float16
EXP = mybir.ActivationFunctionType.Exp
MULT = mybir.AluOpType.mult
ADD = mybir.AluOpType.add

B, T, D, H = 2, 2048, 1024, 16
DH = D // H          # 64
HPC = H // 4         # 4 heads per core
DC = DH * HPC        # 256 channels per core
NBLK = T // 512      # 4 Tq blocks of 512
NT128 = T // 128     # 16 T tiles of 128
NCHUNK = D // 128    # 8 contraction chunks

_PROG = None


def _ensure_axon_hooks():
    """If the runtime sets BASS_TRACE, run_bass_kernel_spmd imports
    antenv.axon_hooks; provide a ctypes-backed NTFF hook when the real
    module isn't shipped (mirrors trn_agent_boot.trn_boot)."""
    try:
        import antenv.axon_hooks  # noqa: F401
        return
    except ImportError:
        pass
    import contextlib
    import ctypes
    import sys
    import types

    try:
        import antenv
    except ImportError:
        antenv = types.ModuleType("antenv")
        sys.modules["antenv"] = antenv

    def _build_hook():
        try:
            lib = ctypes.CDLL("/opt/axon/libaxon_pjrt.so")
        except OSError:
            return None
        if not hasattr(lib, "axon_start_nrt_profile"):
            return None
        lib.axon_start_nrt_profile.argtypes = [
            ctypes.POINTER(ctypes.c_int64),
            ctypes.c_size_t,
        ]
        lib.axon_start_nrt_profile.restype = ctypes.c_int64
        lib.axon_stop_nrt_profile.argtypes = [ctypes.c_char_p]
        lib.axon_stop_nrt_profile.restype = ctypes.c_int64

        @contextlib.contextmanager
        def _ntff_hook(output_dir, device_ids):
            import jax

            jax.devices()
            if device_ids:
                ids = (ctypes.c_int64 * len(device_ids))(*device_ids)
                rc = lib.axon_start_nrt_profile(ids, len(device_ids))
            else:
                rc = lib.axon_start_nrt_profile(None, 0)
            if rc != 0:
                raise RuntimeError(f"axon_start_nrt_profile rc={rc}")
            try:
                yield
            finally:
                n = lib.axon_stop_nrt_profile(str(output_dir).encode())
                if n < 0:
                    raise RuntimeError(f"axon_stop_nrt_profile rc={n}")

        return _ntff_hook

    mod = types.ModuleType("antenv.axon_hooks")
    _cell = {"hook": None, "built": False}

    def set_axon_ntff_profile_hook(hook):
        _cell["hook"] = hook
        _cell["built"] = True

    def get_axon_ntff_profile_hook():
        if not _cell["built"]:
            _cell["hook"] = _build_hook()
            _cell["built"] = True
        return _cell["hook"]

    mod.set_axon_ntff_profile_hook = set_axon_ntff_profile_hook
    mod.get_axon_ntff_profile_hook = get_axon_ntff_profile_hook
    sys.modules["antenv.axon_hooks"] = mod
    antenv.axon_hooks = mod


_ensure_axon_hooks()


def _build_program():
    nc = bacc.Bacc("TRN2", target_bir_lowering=False, debug=False)

    # host-packed inputs: xq/xk[blk, p, c, t] = x^T[128c+p, 512*blk+t]
    #                     xv[s, p, c, t]     = v^T[128c+p, 256*s+t]
    xq = nc.declare_dram_parameter("xq", [NBLK, 128, NCHUNK, 512], F16, isOutput=False)
    xk = nc.declare_dram_parameter("xk", [NBLK, 128, NCHUNK, 512], F16, isOutput=False)
    xv = nc.declare_dram_parameter("xv", [2 * NBLK, 128, NCHUNK, 256], F16, isOutput=False)
    wq = nc.declare_dram_parameter("wq", [128, NCHUNK, DC], F16, isOutput=False)
    wk = nc.declare_dram_parameter("wk", [128, NCHUNK, DC], F16, isOutput=False)
    wv = nc.declare_dram_parameter("wv", [128, NCHUNK, DC], F16, isOutput=False)
    wo = nc.declare_dram_parameter("wo", [128, 2, D], F16, isOutput=False)
    bqk = nc.declare_dram_parameter("bqk", [128, 4], F32, isOutput=False)
    maskp = nc.declare_dram_parameter("maskp", [128, 4, 512], F16, isOutput=False)
    outp = nc.declare_dram_parameter("outp", [NT128, 128, D], F16, isOutput=True)

    with tile.TileContext(nc) as tc, ExitStack() as ctx:
        # one pool per constant: each DMA gets its own semaphore slot so the
        # loads pipeline instead of serializing on completion round-trips
        cp_wv = ctx.enter_context(tc.tile_pool(name="c_wv", bufs=1))
        cp_wk = ctx.enter_context(tc.tile_pool(name="c_wk", bufs=1))
        cp_wq = ctx.enter_context(tc.tile_pool(name="c_wq", bufs=1))
        cp_wo = ctx.enter_context(tc.tile_pool(name="c_wo", bufs=1))
        cp_b = ctx.enter_context(tc.tile_pool(name="c_b", bufs=1))
        cp_m = ctx.enter_context(tc.tile_pool(name="c_m", bufs=1))
        persist = ctx.enter_context(tc.tile_pool(name="persist", bufs=1))
        xkqp = ctx.enter_context(tc.tile_pool(name="xkq", bufs=3))
        xvp = ctx.enter_context(tc.tile_pool(name="xv", bufs=3))
        esp = ctx.enter_context(tc.tile_pool(name="es", bufs=6))
        sop = ctx.enter_context(tc.tile_pool(name="so", bufs=3))
        rdp = ctx.enter_context(tc.tile_pool(name="rd", bufs=2))
        smp = ctx.enter_context(tc.tile_pool(name="small", bufs=1))
        stp = ctx.enter_context(tc.tile_pool(name="st", bufs=2, space="PSUM"))
        xup = ctx.enter_context(tc.tile_pool(name="xu", bufs=1, space="PSUM"))
        pwp = ctx.enter_context(tc.tile_pool(name="pw", bufs=2, space="PSUM"))

        # ---- constants (scalar HWDGE queue), in consumption order ----
        # wv split in half so the first V matmul can start ~2us earlier
        wv_sb = cp_wv.tile([128, NCHUNK, DC], F16)
        nc.scalar.dma_start(wv_sb[:, 0:4, :], wv[:, 0:4, :])
        nc.scalar.dma_start(wv_sb[:, 4:8, :], wv[:, 4:8, :])
        wk_sb = cp_wk.tile([128, NCHUNK, DC], F16)
        nc.scalar.dma_start(wk_sb[:], wk[:])
        wq_sb = cp_wq.tile([128, NCHUNK, DC], F16)
        nc.scalar.dma_start(wq_sb[:], wq[:])
        bqk_sb = cp_b.tile([128, 4], F32)
        nc.scalar.dma_start(bqk_sb[:], bqk[:])
        bq_sb = bqk_sb[:, 0:2]
        bk_sb = bqk_sb[:, 2:4]
        mask_sb = cp_m.tile([128, 4, 512], F16)
        nc.scalar.dma_start(mask_sb[:], maskp[:])
        wo_sb = cp_wo.tile([128, 2, D], F16)
        nc.scalar.dma_start(wo_sb[:], wo[:])

        # preload the exp table set early (one-time ~2.7us); feed it from a
        # memset tile so it does not wait on any DMA
        warm_in = smp.tile([1, 2], F32, tag="warmin")
        nc.vector.memset(warm_in[:], 1.0)
        warm = smp.tile([1, 2], F32, tag="warm")
        nc.scalar.activation(warm[:], warm_in[:], EXP, scale=1.0)

        # persistent activations
        QT = persist.tile([128, 2, T], F16)       # [2 heads x 64dh, pair, T]
        KT = persist.tile([128, 2, T], F16)
        XT = persist.tile([128, 2, T], F16)       # attention out, transposed
        # Vaug[tk, tile, head, 0:64] = 1.0 (denominator rows),
        # Vaug[tk, tile, head, 64:128] = V channels
        Vaug = persist.tile([128, NT128, HPC, 2 * DH], F16)
        nc.vector.memset(Vaug[:], 1.0)

        # ---------------- filler units ----------------
        filler = []  # list of (label, closure); popped strictly FIFO

        def pump(k):
            for _ in range(min(k, len(filler))):
                filler.pop(0)[1]()

        def drain(label):
            # emit every queued unit up to and including the last with `label`
            if not any(lb == label for lb, _ in filler):
                return
            while filler:
                lb, fn = filler.pop(0)
                fn()
                if lb == label and not any(l2 == label for l2, _ in filler):
                    break

        def drain_all():
            while filler:
                filler.pop(0)[1]()

        def v_units(s, vt):
            """V projection for T tiles 2s, 2s+1 -> Vaug[:, :, :, 64:128]."""
            st = {}

            def mk(c):
                def u():
                    if c == 0:
                        # separate psum tiles: start=True clears has_written
                        # BANK-wide, so the two halves must not share a bank
                        st[0] = pwp.tile([128, DC], F32, tag="pw", name="pv0")
                        st[1] = pwp.tile([128, DC], F32, tag="pw", name="pv1")
                    for k in (0, 1):
                        nc.tensor.matmul(
                            st[k][:], vt[:, c, 128 * k : 128 * (k + 1)],
                            wv_sb[:, c, :],
                            start=(c == 0), stop=(c == NCHUNK - 1),
                            skip_group_check=True,
                        )
                return u

            def fin():
                for k in (0, 1):
                    t = 2 * s + k
                    nc.vector.tensor_copy(
                        Vaug[:, t, :, DH : 2 * DH],
                        st[k][:].rearrange("p (h d) -> p h d", h=HPC),
                    )

            return [mk(c) for c in range(NCHUNK)] + [fin]

        def kq_units(xt, w_sb, b_sb, OUT, blk):
            """K^T or Q^T projection for Tq block blk (both pairs)."""
            st = {}

            def mk(c):
                def u():
                    if c == 0:
                        st[0] = pwp.tile([128, 512], F32, tag="pw", name="ps0")
                        st[1] = pwp.tile([128, 512], F32, tag="pw", name="ps1")
                    for p in (0, 1):
                        nc.tensor.matmul(
                            st[p][:], w_sb[:, c, 128 * p : 128 * (p + 1)],
                            xt[:, c, :],
                            start=(c == 0), stop=(c == NCHUNK - 1),
                            skip_group_check=True,
                        )
                return u

            def fin():
                for p in (0, 1):
                    nc.vector.tensor_scalar(
                        OUT[:, p, 512 * blk : 512 * (blk + 1)],
                        st[p][:], b_sb[:, p : p + 1], None, op0=ADD,
                    )

            return [mk(c) for c in range(NCHUNK)] + [fin]

        def wo_units(blk, copy_eng=None):
            """Output projection + store for the 4 T tiles of block blk."""
            st = {}
            units = []
            for t in range(4 * blk, 4 * blk + 4):
                for n in (0, 1):
                    def u(t=t, n=n):
                        if n == 0:
                            st[t] = sop.tile([128, D], F16, tag="so", name="so")
                        po = pwp.tile([128, 512], F32, tag="pw", name="po")
                        nc.tensor.matmul(
                            po[:], XT[:, 0, 128 * t : 128 * (t + 1)],
                            wo_sb[:, 0, 512 * n : 512 * (n + 1)],
                            start=True, stop=False, skip_group_check=True,
                        )
                        nc.tensor.matmul(
                            po[:], XT[:, 1, 128 * t : 128 * (t + 1)],
                            wo_sb[:, 1, 512 * n : 512 * (n + 1)],
                            start=False, stop=True, skip_group_check=True,
                        )
                        dst = st[t][:, 512 * n : 512 * (n + 1)]
                        if copy_eng == "scalar" or (copy_eng == "alt" and n == 1):
                            nc.scalar.copy(dst, po[:])
                        else:
                            nc.vector.tensor_copy(dst, po[:])
                        if n == 1:
                            nc.gpsimd.dma_start(outp[t], st[t][:])
                    units.append(u)
            return units

        # ---------------- attention ----------------
        last_es = [None]

        def att_tile(p, i, j, njt, xu):
            """One 128(tk) x 512(tq) score tile for head pair p, block i."""
            J = j - 4 * i
            c0 = 128 * J if J >= 0 else 0
            ps_t = stp.tile([128, 2, 512], F32, tag="st", name="ps_t")
            for hp in range(2):
                nc.tensor.matmul(
                    ps_t[:, hp, c0:512],
                    KT[64 * hp : 64 * hp + 64, p, 128 * j : 128 * (j + 1)],
                    QT[64 * hp : 64 * hp + 64, p, 512 * i + c0 : 512 * (i + 1)],
                    start=True, stop=True, skip_group_check=True,
                )
            es = esp.tile([128, 2, 512], F16, tag="es", name="es")
            nc.scalar.activation(
                es[:, :, c0:512], ps_t[:, :, c0:512], EXP,
                scale=1.0 / np.sqrt(DH),
            )
            if J >= 0:
                for hp in range(2):
                    nc.vector.tensor_tensor(
                        es[:, hp, c0:512], es[:, hp, c0:512],
                        mask_sb[:, J, c0:512], op=MULT,
                    )
            for hp in range(2):
                nc.tensor.matmul(
                    xu[:, hp, c0:512], Vaug[:, j, 2 * p + hp, :],
                    es[:, hp, c0:512],
                    start=(j == 0), stop=(j == njt - 1),
                    skip_group_check=True,
                )
            last_es[0] = es

        def att_normalize(p, i, xu):
            rd = rdp.tile([DH, 2, 512], F32, tag="rd", name="rd")
            nc.vector.reciprocal_approx_fast(rd[:], xu[0:DH, :, :])
            for hp in range(2):
                nc.vector.tensor_tensor(
                    XT[64 * hp : 64 * hp + 64, p, 512 * i : 512 * (i + 1)],
                    xu[DH:128, hp, :], rd[:, hp, :], op=MULT,
                )

        # ---------------- prologue ----------------
        # All x DMAs upfront on the sync HWDGE queue, in consumption order.
        # The bufs=2 pools gate them: block b's trigger fires once block
        # b-2's tiles have been consumed (self-pacing flow control).
        vts = {}
        kts = {}
        qts = {}
        vts[0] = xvp.tile([128, NCHUNK, 256], F16, tag="xv", name="vt")
        nc.sync.dma_start(vts[0][:, 0:4, :], xv[0][:, 0:4, :])
        nc.sync.dma_start(vts[0][:, 4:8, :], xv[0][:, 4:8, :])
        vts[1] = xvp.tile([128, NCHUNK, 256], F16, tag="xv", name="vt")
        nc.sync.dma_start(vts[1][:], xv[1])
        kts[0] = xkqp.tile([128, NCHUNK, 512], F16, tag="xk", name="kt")
        nc.sync.dma_start(kts[0][:, 0:4, :], xk[0][:, 0:4, :])
        nc.sync.dma_start(kts[0][:, 4:8, :], xk[0][:, 4:8, :])
        qts[0] = xkqp.tile([128, NCHUNK, 512], F16, tag="xq", name="qt")
        nc.sync.dma_start(qts[0][:, 0:4, :], xq[0][:, 0:4, :])
        nc.sync.dma_start(qts[0][:, 4:8, :], xq[0][:, 4:8, :])
        for b in (1, 2, 3):
            for s in (2 * b, 2 * b + 1):
                vts[s] = xvp.tile([128, NCHUNK, 256], F16, tag="xv", name="vt")
                nc.sync.dma_start(vts[s][:], xv[s])
            kts[b] = xkqp.tile([128, NCHUNK, 512], F16, tag="xk", name="kt")
            nc.sync.dma_start(kts[b][:], xk[b])
            qts[b] = xkqp.tile([128, NCHUNK, 512], F16, tag="xq", name="qt")
            nc.sync.dma_start(qts[b][:], xq[b])

        def proj_units(blk):
            return (
                [("proj%d" % blk, u) for u in v_units(2 * blk, vts[2 * blk])]
                + [("proj%d" % blk, u) for u in v_units(2 * blk + 1, vts[2 * blk + 1])]
                + [("proj%d" % blk, u) for u in kq_units(kts[blk], wk_sb, bk_sb, KT, blk)]
                + [("proj%d" % blk, u) for u in kq_units(qts[blk], wq_sb, bq_sb, QT, blk)]
            )

        # proj(0) drains inline (nothing else to do yet)
        for _, u in proj_units(0):
            u()
        filler.extend(proj_units(1))

        # ---------------- main loop over Tq blocks ----------------
        # filler placement: proj(i+1) during block i; wo(0) in b1 (XT(0)
        # ready), wo(1)+wo(2) deferred to b3 (the ACT-paced block with the
        # least projection filler)
        # wo(2) units double as pair-boundary cover in block 3: emitted
        # BEFORE each normalize (units emitted after a normalize inherit a
        # dependency on it through XT and cannot cover it), with psum->sbuf
        # copies on ScalarE so the normalize chain is not delayed on DVE
        wo2_cover = wo_units(2, copy_eng="scalar")

        for i in range(NBLK):
            if i == 1:
                filler.extend(proj_units(2))
                filler.extend([("wo0", u) for u in wo_units(0)])
            elif i == 2:
                filler.extend(proj_units(3))
            elif i == 3:
                filler.extend([("wo1", u) for u in wo_units(1)])

            # everything block i consumes must be emitted by now
            drain("proj%d" % i)

            njt = 4 * i + 4
            ntile = 2 * njt
            for p in (0, 1):
                xu = xup.tile([128, 2, 512], F32, tag="xu", name="xu")
                for j in range(njt):
                    att_tile(p, i, j, njt, xu)
                    # spread remaining filler over remaining attention tiles,
                    # holding back ~3 units per upcoming pair boundary to
                    # cover the normalize -> next-pair-PV psum dependency
                    rem_tiles = ntile - (p * njt + j + 1)
                    if rem_tiles > 0:
                        reserve = 3 * (2 - p)
                        avail = max(0, len(filler) - reserve)
                        need = (avail + rem_tiles - 1) // rem_tiles
                        pump(need)
                # boundary cover: emitted BEFORE the normalize so the PE
                # stays busy while the DVE normalize chain runs
                if i == 3:
                    ncov = 4 if p == 0 else len(wo2_cover)
                    for u in wo2_cover[:ncov]:
                        u()
                    del wo2_cover[:ncov]
                else:
                    pump(3)
                att_normalize(p, i, xu)
                if i == 3 and p == 1:
                    # HAM keep-warm: the final normalize leaves the PE with
                    # no legal work for ~2.7us, which re-throttles the clock
                    # and makes the whole wo(3) tail run at 1.2 GHz.  Pin
                    # dummy matmuls here — gated on the last es tile so the
                    # list scheduler cannot hoist them earlier.
                    for _ in range(5):
                        dps = stp.tile([128, 2, 512], F32, tag="st", name="dummy")
                        nc.tensor.matmul(
                            dps[:, 0, :], Vaug[:, 0, 0, :], last_es[0][:, 0, :],
                            start=True, stop=True, skip_group_check=True,
                        )

        drain_all()
        # tail: alternate the psum->sbuf copies DVE/ACT (ACT is idle here)
        for u in wo_units(3, copy_eng="alt"):
            u()

    nc.compile()
    return nc


def _get_program():
    global _PROG
    if _PROG is None:
        _PROG = _build_program()
    return _PROG


def _make_mask():
    r = np.arange(128)[:, None]
    c = np.arange(512)[None, :]
    m = np.zeros((128, 4, 512), np.float16)
    for J in range(4):
        m[:, J, :] = (c >= 128 * J + r).astype(np.float16)
    return m


def _core_inputs(inputs, b, g):
    """Per-core input map (host-side sharding/layout prep)."""
    f = np.float32
    sl = slice(DC * g, DC * (g + 1))
    wqp = np.ascontiguousarray(
        np.asarray(inputs["Wq"], f)[:, sl].reshape(NCHUNK, 128, DC).transpose(1, 0, 2)
    ).astype(np.float16)
    wkp = np.ascontiguousarray(
        np.asarray(inputs["Wk"], f)[:, sl].reshape(NCHUNK, 128, DC).transpose(1, 0, 2)
    ).astype(np.float16)
    wvp = np.ascontiguousarray(
        np.asarray(inputs["Wv"], f)[:, sl].reshape(NCHUNK, 128, DC).transpose(1, 0, 2)
    ).astype(np.float16)
    wop = np.ascontiguousarray(
        np.asarray(inputs["Wo"], f)[sl, :].reshape(2, 128, D).transpose(1, 0, 2)
    ).astype(np.float16)
    bqp = np.asarray(inputs["bq"], f)[sl].reshape(2, 128).T
    bkp = np.asarray(inputs["bk"], f)[sl].reshape(2, 128).T
    return {
        "wq": wqp, "wk": wkp, "wv": wvp, "wo": wop,
        "bqk": np.ascontiguousarray(np.concatenate([bqp, bkp], axis=1)),
        "maskp": _make_mask(),
    }


def _pack_x(xt_f16, inner):
    """[1024, 2048] fp16 x^T -> [T//inner, 128, 8, inner] host-packed."""
    return np.ascontiguousarray(
        xt_f16.reshape(NCHUNK, 128, T // inner, inner).transpose(2, 1, 0, 3)
    )


def run_cores(inputs, trace=False, trace_cores=None):
    nc = _get_program()
    f = np.float32
    xb = {}
    for b in range(B):
        qT = np.asarray(inputs["q"], f)[b].T.astype(np.float16)
        kT = np.asarray(inputs["k"], f)[b].T.astype(np.float16)
        vT = np.asarray(inputs["v"], f)[b].T.astype(np.float16)
        xb[b] = {
            "xq": _pack_x(qT, 512),
            "xk": _pack_x(kT, 512),
            "xv": _pack_x(vT, 256),
        }
    in_maps = []
    for c in range(8):
        b, g = divmod(c, 4)
        m = _core_inputs(inputs, b, g)
        m.update(xb[b])
        in_maps.append(m)
    kw = {}
    if trace:
        kw = dict(trace=True, trace_cores=trace_cores or [0])
    res = run_bass_kernel_spmd(nc, in_maps, list(range(8)), **kw)
    bo = np.asarray(inputs["bo"], f)
    out = np.empty((B, T, D), f)
    for b in range(B):
        acc = res.results[4 * b]["outp"].astype(f)
        for g in range(1, 4):
            acc = acc + res.results[4 * b + g]["outp"].astype(f)
        out[b] = acc.reshape(T, D) + bo
    return out, res


def kernel(**inputs) -> np.ndarray:
    out, _ = run_cores(inputs)
    return out
